# revision 1
# baseline (speedup 1.0000x reference)
"""Talking-heads attention on 8 Trainium2 NeuronCores.

Sharding: data-parallel over (batch b in 0..3) x (query half in 0..1) -> 8 cores.
Each core computes K/V for its full batch sequence (1024) and attention for its
512 query rows. No collectives.

Math notes (per core, all layouts transposed so contractions sit on partitions):
  - mix_pre is folded into Q: qs_g[hd, i] = qT[hd, i] * SCALE * mix_pre[h(hd), g],
    so dotsT_g[j, i] = sum_hd kT[hd, j] * qs_g[hd, i] over the full 768 dim.
  - softmax over j (partitions) without max-subtraction (|dots| <~ 6, safe in f32);
    S_g[i] = sum_j exp via ones-matmul, 1/S via exp(-ln S) on ACT.
  - mix_post is folded into V: Vt_g[j, (g',d)] = mix_post[g, g'] * v[j, (g',d)];
    out2T[(g'd), i] += sum_j Vt_g[j, gd] * attnT_g[j, i] accumulated in PSUM over g.
  - out = out2T.T @ Wout + bout.
"""

import numpy as np

import concourse.bass as bass
import concourse.mybir as mybir
import concourse.tile as tile
from concourse import bacc
from concourse.bass_utils import run_bass_kernel_spmd

P = 128
DIM = 768
SEQ = 1024
IQ = 512            # query rows per core
H = 12
DH = 64
NC6 = DIM // P      # 6 chunks of the 768 dim
JC8 = SEQ // P      # 8 chunks of the key dim
SCALE = DH ** -0.5
F32 = mybir.dt.float32
BF16 = mybir.dt.bfloat16

_CACHE = {}


def _build_nc():
    nc = bacc.Bacc("TRN2", target_bir_lowering=False, debug=False)

    xqT = nc.dram_tensor("xqT", [DIM, IQ], BF16, kind="ExternalInput")
    xkvT = nc.dram_tensor("xkvT", [DIM, SEQ], BF16, kind="ExternalInput")
    Wq = nc.dram_tensor("Wq", [DIM, DIM], BF16, kind="ExternalInput")
    Wk = nc.dram_tensor("Wk", [DIM, DIM], BF16, kind="ExternalInput")
    Wv = nc.dram_tensor("Wv", [DIM, DIM], BF16, kind="ExternalInput")
    Wout = nc.dram_tensor("Wout", [DIM, DIM], F32, kind="ExternalInput")
    bout = nc.dram_tensor("bout", [1, DIM], F32, kind="ExternalInput")
    mixpre = nc.dram_tensor("mixpre", [H, H], F32, kind="ExternalInput")
    mixpostT = nc.dram_tensor("mixpostT", [H, H], F32, kind="ExternalInput")
    out = nc.dram_tensor("out", [IQ, DIM], F32, kind="ExternalOutput")

    r3 = lambda t: t.rearrange("(c p) e -> p c e", p=P)

    with tile.TileContext(nc) as tc:
        with (
            tc.tile_pool(name="persist", bufs=1) as pp,
            tc.tile_pool(name="consts", bufs=1) as cp,
        ):
            # ---- constants ----
            # head indicator E[p, col] = 1.0 iff col // 64 == p  (kron(I12, ones64))
            E = cp.tile([H, DIM], F32)
            nc.gpsimd.memset(E[:], 1.0)
            nc.gpsimd.affine_select(
                out=E[:], in_=E[:], compare_op=mybir.AluOpType.is_ge, fill=0.0,
                base=0, pattern=[[1, DIM]], channel_multiplier=-DH,
            )
            nc.gpsimd.affine_select(
                out=E[:], in_=E[:], compare_op=mybir.AluOpType.is_ge, fill=0.0,
                base=DH - 1, pattern=[[-1, DIM]], channel_multiplier=DH,
            )
            ones128b = cp.tile([P, 1], BF16)
            nc.gpsimd.memset(ones128b[:], 1.0)
            ones1_128 = cp.tile([1, P], F32)
            nc.gpsimd.memset(ones1_128[:], 1.0)
            ones12_128 = cp.tile([H, P], F32)
            nc.gpsimd.memset(ones12_128[:], 1.0)
            mixpre_sb = cp.tile([H, H], F32)
            nc.gpsimd.dma_start(mixpre_sb[:], mixpre[:])
            mixpostT_sb = cp.tile([H, H], F32)
            nc.gpsimd.dma_start(mixpostT_sb[:], mixpostT[:])
            bout_sb = cp.tile([1, DIM], F32)
            nc.gpsimd.dma_start(bout_sb[:], bout[:])

            # ---- persistent activations ----
            qT = pp.tile([P, NC6, IQ], BF16)      # scaled by SCALE at copy
            kT = pp.tile([P, NC6, SEQ], BF16)
            V = pp.tile([P, JC8, DIM], BF16)     # [j-part, jc, (g,d)]
            Wout_sb = pp.tile([P, NC6, DIM], F32)
            scaleT = pp.tile([P, NC6, H], F32)   # mix_pre expanded to hd rows
            bout_t = pp.tile([P, DIM], F32)      # bout broadcast to all partitions
            o2_sb = pp.tile([P, NC6, IQ], F32)   # out2T staged for out-proj

            nc.gpsimd.dma_start(Wout_sb[:], r3(Wout))

            # ---- phase 1: projections ----
            with (
                tc.tile_pool(name="pin", bufs=1) as pin,
                tc.tile_pool(name="pj", bufs=2, space="PSUM") as pj,
                tc.tile_pool(name="pjsm", bufs=2, space="PSUM") as pjsm,
            ):
                xqT_sb = pin.tile([P, NC6, IQ], BF16)
                xkvT_sb = pin.tile([P, NC6, SEQ], BF16)
                Wq_sb = pin.tile([P, NC6, DIM], BF16)
                Wk_sb = pin.tile([P, NC6, DIM], BF16)
                Wv_sb = pin.tile([P, NC6, DIM], BF16)
                nc.gpsimd.dma_start(xqT_sb[:], r3(xqT))
                nc.gpsimd.dma_start(xkvT_sb[:], r3(xkvT))
                nc.gpsimd.dma_start(Wq_sb[:], r3(Wq))
                nc.gpsimd.dma_start(Wk_sb[:], r3(Wk))
                nc.gpsimd.dma_start(Wv_sb[:], r3(Wv))

                # qT[e,i] = sum_f Wq[f,e] xqT[f,i] ; folds SCALE on the copy out
                for ec in range(NC6):
                    ps = pj.tile([P, IQ], F32, tag="pjq")
                    for fc in range(NC6):
                        nc.tensor.matmul(
                            ps[:], Wq_sb[:, fc, ec * P : (ec + 1) * P],
                            xqT_sb[:, fc, :], start=(fc == 0), stop=(fc == NC6 - 1),
                        )
                    nc.vector.tensor_scalar_mul(qT[:, ec, :], ps[:], SCALE)

                # kT[e,j]
                for ec in range(NC6):
                    for jh in range(2):
                        ps = pj.tile([P, IQ], F32, tag="pjq")
                        for fc in range(NC6):
                            nc.tensor.matmul(
                                ps[:], Wk_sb[:, fc, ec * P : (ec + 1) * P],
                                xkvT_sb[:, fc, jh * IQ : (jh + 1) * IQ],
                                start=(fc == 0), stop=(fc == NC6 - 1),
                            )
                        nc.vector.tensor_copy(kT[:, ec, jh * IQ : (jh + 1) * IQ], ps[:])

                # V[j, gd] = sum_f xkvT[f, j] Wv[f, gd]  (row-major j on partitions)
                for jc in range(JC8):
                    ps = pjsm.tile([P, DIM], F32, tag="pjv")
                    for ns, ne in ((0, 512), (512, DIM)):
                        for fc in range(NC6):
                            nc.tensor.matmul(
                                ps[:, ns:ne],
                                xkvT_sb[:, fc, jc * P : (jc + 1) * P],
                                Wv_sb[:, fc, ns:ne],
                                start=(fc == 0), stop=(fc == NC6 - 1),
                            )
                    nc.scalar.copy(V[:, jc, :], ps[:])

                # scaleT[p, c, g] = mix_pre[h(c,p), g]
                for c in range(NC6):
                    ps = pj.tile([P, H], F32, tag="pjsc")
                    nc.tensor.matmul(
                        ps[:], E[:, c * P : (c + 1) * P], mixpre_sb[:],
                        start=True, stop=True,
                    )
                    nc.vector.tensor_copy(scaleT[:, c, :], ps[:])

                # bout broadcast to [128, DIM]
                psb = pjsm.tile([P, DIM], F32, tag="pjv")
                for ns, ne in ((0, 512), (512, DIM)):
                    nc.tensor.matmul(
                        psb[:, ns:ne], ones1_128[:], bout_sb[:, ns:ne],
                        start=True, stop=True,
                    )
                nc.vector.tensor_copy(bout_t[:], psb[:])

            # ---- phase 2: attention with both mixes folded ----
            with (
                tc.tile_pool(name="acc", bufs=1, space="PSUM") as acc,
                tc.tile_pool(name="pwork", bufs=2, space="PSUM") as pwork,
                tc.tile_pool(name="gbufs", bufs=2) as gb,
                tc.tile_pool(name="small", bufs=2) as sp,
            ):
                o2ps = [
                    acc.tile([P, IQ], F32, tag=f"o2_{s}", name=f"o2_{s}")
                    for s in range(NC6)
                ]

                for g in range(H):
                    # q-scale: fold mix_pre column g
                    qs = gb.tile([P, NC6, IQ], BF16, tag="qs")
                    for c in range(NC6):
                        nc.vector.tensor_scalar_mul(
                            qs[:, c, :], qT[:, c, :], scaleT[:, c, g : g + 1]
                        )
                    # V-scale: fold mix_post row g -> Vt_g
                    D = sp.tile([H, DIM], F32, tag="D")
                    nc.vector.tensor_scalar_mul(D[:], E[:], mixpostT_sb[:, g : g + 1])
                    vscale = sp.tile([P, DIM], BF16, tag="vscale")
                    for ns, ne in ((0, 512), (512, DIM)):
                        vs_ps = pwork.tile([P, ne - ns], F32, tag="work", name="vs_ps")
                        nc.tensor.matmul(
                            vs_ps[:], ones12_128[:], D[:, ns:ne],
                            start=True, stop=True,
                        )
                        nc.scalar.copy(vscale[:, ns:ne], vs_ps[:])
                    Vt = gb.tile([P, JC8, DIM], BF16, tag="Vt")
                    nc.vector.tensor_tensor(
                        Vt[:], V[:], vscale[:, None, :].to_broadcast((P, JC8, DIM)),
                        mybir.AluOpType.mult,
                    )

                    # dots + exp
                    attnT = gb.tile([P, JC8, IQ], BF16, tag="attnT")
                    for jc in range(JC8):
                        ds = pwork.tile([P, IQ], F32, tag="work")
                        for c in range(NC6):
                            nc.tensor.matmul(
                                ds[:], kT[:, c, jc * P : (jc + 1) * P], qs[:, c, :],
                                start=(c == 0), stop=(c == NC6 - 1),
                            )
                        nc.scalar.activation(
                            attnT[:, jc, :], ds[:], mybir.ActivationFunctionType.Exp
                        )

                    # S and 1/S
                    S_ps = pwork.tile([1, IQ], F32, tag="work")
                    for jc in range(JC8):
                        nc.tensor.matmul(
                            S_ps[:], ones128b[:], attnT[:, jc, :],
                            start=(jc == 0), stop=(jc == JC8 - 1),
                        )
                    rS = sp.tile([1, IQ], F32, tag="rS")
                    rscr = sp.tile([1, IQ], F32, tag="rscr")
                    nc.vector.reciprocal_approx_accurate(
                        out=rS[:], in_=S_ps[:], scratch=rscr[:]
                    )
                    R_ps = pwork.tile([P, IQ], F32, tag="work")
                    nc.tensor.matmul(R_ps[:], ones1_128[:], rS[:], start=True, stop=True)
                    R = sp.tile([P, IQ], BF16, tag="R")
                    nc.scalar.copy(R[:], R_ps[:])
                    for jc in range(JC8):
                        nc.vector.tensor_tensor(
                            attnT[:, jc, :], attnT[:, jc, :], R[:],
                            mybir.AluOpType.mult,
                        )

                    # out2T accumulation over g
                    for s in range(NC6):
                        for jc in range(JC8):
                            nc.tensor.matmul(
                                o2ps[s][:],
                                Vt[:, jc, s * P : (s + 1) * P],
                                attnT[:, jc, :],
                                start=(g == 0 and jc == 0),
                                stop=(g == H - 1 and jc == JC8 - 1),
                            )

                for s in range(NC6):
                    nc.vector.tensor_copy(o2_sb[:, s, :], o2ps[s][:])

            # ---- phase 3: output projection + bias ----
            with (
                tc.tile_pool(name="pj3", bufs=2, space="PSUM") as pj3,
                tc.tile_pool(name="oBuf", bufs=2) as ob,
            ):
                for isl in range(IQ // P):
                    fp = pj3.tile([P, DIM], F32, tag="fin")
                    for ns, ne in ((0, 512), (512, DIM)):
                        for c in range(NC6):
                            nc.tensor.matmul(
                                fp[:, ns:ne],
                                o2_sb[:, c, isl * P : (isl + 1) * P],
                                Wout_sb[:, c, ns:ne],
                                start=(c == 0), stop=(c == NC6 - 1),
                            )
                    osb = ob.tile([P, DIM], F32, tag="osb")
                    nc.vector.tensor_tensor(
                        osb[:], fp[:], bout_t[:], mybir.AluOpType.add
                    )
                    nc.gpsimd.dma_start(out[isl * P : (isl + 1) * P, :], osb[:])

    nc.compile()
    return nc


def kernel(x, Wq, Wkv, mix_pre, mix_post, Wout, bout):
    x = np.asarray(x, dtype=np.float32)
    Wq = np.asarray(Wq, dtype=np.float32)
    Wkv = np.asarray(Wkv, dtype=np.float32)
    mix_pre = np.asarray(mix_pre, dtype=np.float32)
    mix_post = np.asarray(mix_post, dtype=np.float32)
    Wout = np.asarray(Wout, dtype=np.float32)
    bout = np.asarray(bout, dtype=np.float32)

    if "nc" not in _CACHE:
        _CACHE["nc"] = _build_nc()
    nc = _CACHE["nc"]

    import ml_dtypes
    bf = ml_dtypes.bfloat16
    Wk = np.ascontiguousarray(Wkv[:, :DIM]).astype(bf)
    Wv = np.ascontiguousarray(Wkv[:, DIM:]).astype(bf)
    shared = {
        "Wq": Wq.astype(bf), "Wk": Wk, "Wv": Wv, "Wout": Wout,
        "bout": np.ascontiguousarray(bout.reshape(1, DIM)),
        "mixpre": mix_pre,
        "mixpostT": np.ascontiguousarray(mix_post.T),
    }
    b_, n_, d_ = x.shape
    in_maps = []
    for c in range(8):
        b, half = c // 2, c % 2
        m = dict(shared)
        m["xqT"] = np.ascontiguousarray(x[b, half * IQ : (half + 1) * IQ, :].T).astype(bf)
        m["xkvT"] = np.ascontiguousarray(x[b].T).astype(bf)
        in_maps.append(m)

    res = run_bass_kernel_spmd(nc, in_maps, core_ids=list(range(8)))
    _CACHE["last_results"] = res

    full = np.empty((b_, n_, d_), dtype=np.float32)
    for c in range(8):
        b, half = c // 2, c % 2
        full[b, half * IQ : (half + 1) * IQ, :] = res.results[c]["out"]
    return full

